# revision 5
# baseline (speedup 1.0000x reference)
"""ContourLoss on 8 Trainium2 NeuronCores (data parallel over batch B=8).

Device work per core (one sample):
  - Intersection grid over compacted valid points: for segment pairs i,j
        d1*d2 = <V(i), U(j)>,  d3*d4 = <U(i), V(j)>
    with U/V 6 quadratic per-segment features -> two small-K matmuls per
    [128 x N] tile on the tensor engine.  Per-sample validity (i,j < n_seg)
    is folded into two extra feature rows adding +BIG to q12 so
    sigmoid(-0.01*q12) underflows to exactly 0.
  - sigmoid(-0.01*q) on the scalar engine (scale fused), product on the
    vector engine, per-tile row-sum into a private column of a [128, 64]
    partials tile (no cross-tile serialization), host sums columns.
  - Triangular mask (j >= i+2) only affects the leading 132-wide chunk of
    each 128-row band; handled there with a constant 0/1 tile.
  - CE / SmoothL1 / cosine terms are small [128, <=64] elementwise work.
The host does the O(B*S) prep (compaction order, masks, features) and the
final scalar arithmetic; denominators/counts and the reference's excluded
wrap pair (i=0, j=n_seg-1) are computed host-side.
"""

import numpy as np

RETINA = 224.0
NUM_CLASSES = 4
B = 8
S = 2048
M = S - 1
NCORES = 8
W_DIAG = 132  # leading chunk per 128-band: covers all cells with j-i < 2
BIG = 1.0e13
NCOLS = 64    # partials tile width: 5 cheap cols + per-block isect cols
ISECT_COL0 = 5

_CACHE = {}
TRACE_KWARGS = {}  # test harness sets e.g. {"trace": True} to profile
LAST_RESULTS = None


# ---------------------------------------------------------------------------
# walrus in this environment accepts at most ONE sync-wait per instruction;
# the pinned concourse Tile stack can attach several (notably the kernel-tail
# Drain).  Splitting extras onto same-engine NoOps is semantically identical.
def _split_multi_waits(nc, max_waits=1):
    import concourse.mybir as mybir
    n_split = 0
    for fn in nc.m.functions:
        for blk in fn.blocks:
            out = []
            changed = False
            for inst in blk.instructions:
                si = inst.sync_info
                ow = list(si.on_wait) if (si is not None and si.on_wait) else []
                if len(ow) > max_waits:
                    for k, w in enumerate(ow[:-max_waits]):
                        out.append(mybir.InstNoOp(
                            name=f"{inst.name}_wsplit{k}",
                            engine=inst.engine,
                            ins=[], outs=[],
                            sync_info=mybir.SyncInfo(on_wait=[w],
                                                     on_update=[]),
                        ))
                        n_split += 1
                    si.on_wait = ow[-max_waits:]
                    changed = True
                out.append(inst)
            if changed:
                blk.instructions = out
    return n_split


def _schedule(L):
    """Chunk list [(i0, j0, N, first), ...]; `first` chunks are triangular."""
    chunks = []
    for ib in range(L // 128):
        i0 = 128 * ib
        j0 = i0
        first = True
        while j0 < L:
            N = min(W_DIAG if first else 512, L - j0)
            chunks.append((i0, j0, N, first))
            j0 += N
            first = False
    return chunks


def _host_prep(pp, op, cp, ts, pm):
    """Per-sample compaction + feature construction (all O(B*S))."""
    tc_cls = ts[:, :, 4].astype(np.int32)
    tp = ts[:, :, :2]
    to = ts[:, :, 2:4]
    valid = ~pm
    nn = valid & (tc_cls != 0)

    per_core = []
    n_segs = []
    for b in range(B):
        order = np.argsort(~nn[b], kind="stable")
        pts = pp[b][order].astype(np.float64)
        n = int(nn[b].sum())
        n_seg = n - 1
        n_segs.append(n_seg)
        if n > 0:
            pts = pts - pts[:n].mean(axis=0)
        sx, sy = pts[:-1, 0], pts[:-1, 1]
        eX, eY = pts[1:, 0], pts[1:, 1]
        ex, ey = eX - sx, eY - sy
        c = ex * sy - ey * sx
        g0, g1, g2 = ex, -ey, -c
        one = np.ones(M)
        # f1 = (sy, sx, 1), f2 = (eY, eX, 1)
        U6 = np.stack([g0 * g0, g1 * g1, g2 * g2,
                       g0 * g1, g0 * g2, g1 * g2], 0)
        V6 = np.stack([sy * eY, sx * eX, one,
                       sy * eX + sx * eY,
                       sy + eY,
                       sx + eX], 0)
        inv = (np.arange(M) >= max(n_seg, 0)).astype(np.float64) * BIG
        fA12 = np.concatenate([V6, inv[None], one[None]], 0)  # [8, M]
        fB12 = np.concatenate([U6, one[None], inv[None]], 0)  # [8, M]
        per_core.append(dict(n=n, n_seg=n_seg,
                             fA12=fA12, fB12=fB12, fA34=U6, fB34=V6))
    return tc_cls, tp, to, valid, nn, per_core, n_segs


def _build_program(L):
    import concourse.bass as bass
    import concourse.tile as tile
    from concourse import mybir

    f32 = mybir.dt.float32
    ALU = mybir.AluOpType
    ACT = mybir.ActivationFunctionType
    AX = mybir.AxisListType

    sched = _schedule(L)
    assert ISECT_COL0 + len(sched) <= NCOLS, (L, len(sched))

    nc = bass.Bass()
    # packed features: [8, 4*L] = fA12 | fB12 | fA34(rows 0:6) | fB34(rows 0:6)
    d_feat = nc.dram_tensor("feat", [8, 4 * L], f32, kind="ExternalInput")
    d_tri = nc.dram_tensor("tri", [128, W_DIAG], f32, kind="ExternalInput")
    # packed per-token data: ppn|tpn|opr|ton|cp4|ohv|vf
    d_tok = nc.dram_tensor("tok", [128, 272], f32, kind="ExternalInput")
    d_out = nc.dram_tensor("partials", [128, NCOLS], f32,
                           kind="ExternalOutput")

    with tile.TileContext(nc) as tc:
        with (
            tc.tile_pool(name="singles", bufs=1) as singles,
            tc.tile_pool(name="sig", bufs=3) as sig,
            tc.tile_pool(name="psum", bufs=2, space="PSUM") as psum,
        ):
            feat = singles.tile([8, 4 * L], f32)
            tri = singles.tile([128, W_DIAG], f32)
            tok = singles.tile([128, 272], f32)
            nc.sync.dma_start(out=feat[:], in_=d_feat[:])
            nc.sync.dma_start(out=tok[:], in_=d_tok[:])
            nc.sync.dma_start(out=tri[:], in_=d_tri[:])

            fA12 = feat[:, 0 * L:1 * L]
            fB12 = feat[:, 1 * L:2 * L]
            fA34 = feat[0:6, 2 * L:3 * L]
            fB34 = feat[0:6, 3 * L:4 * L]
            ppn = tok[:, 0:32]
            tpn = tok[:, 32:64]
            opr = tok[:, 64:96]
            ton = tok[:, 96:128]
            cp4 = tok[:, 128:192]
            ohv = tok[:, 192:256]
            vf = tok[:, 256:272]

            cols = singles.tile([128, NCOLS], f32)
            junk = singles.tile([128, 512], f32)
            dpt = singles.tile([128, 32], f32)
            e4 = singles.tile([128, 64], f32)
            gs = singles.tile([128, 16], f32)
            lg = singles.tile([128, 16], f32)

            nc.vector.memset(cols, 0.0)

            # ---- cheap losses (ACT: Exp then Ln, before the sigmoid set) ---
            # col0: sum (pp-tp)^2 * nn   (host scales by 0.25/RET^2)
            nc.vector.tensor_tensor(out=dpt[:], in0=ppn, in1=tpn,
                                    op=ALU.subtract)
            nc.vector.tensor_tensor(out=junk[:, :32], in0=dpt[:], in1=dpt[:],
                                    op=ALU.mult)
            nc.vector.tensor_reduce(out=cols[:, 0:1], in_=junk[:, :32],
                                    axis=AX.X, op=ALU.add)
            # col1: sum (op . to) * nn
            nc.vector.tensor_tensor(out=junk[:, :32], in0=opr, in1=ton,
                                    op=ALU.mult)
            nc.vector.tensor_reduce(out=cols[:, 1:2], in_=junk[:, :32],
                                    axis=AX.X, op=ALU.add)
            # col2: sum lse*vf ; col3: sum x_sel*vf
            nc.scalar.activation(out=e4[:], in_=cp4, func=ACT.Exp)
            nc.vector.tensor_reduce(
                out=gs[:], in_=e4[:].rearrange("p (t c) -> p t c", c=4),
                axis=AX.X, op=ALU.add)
            nc.scalar.activation(out=lg[:], in_=gs[:], func=ACT.Ln)
            nc.vector.tensor_tensor(out=junk[:, :16], in0=lg[:], in1=vf,
                                    op=ALU.mult)
            nc.vector.tensor_reduce(out=cols[:, 2:3], in_=junk[:, :16],
                                    axis=AX.X, op=ALU.add)
            nc.vector.tensor_tensor(out=junk[:, :64], in0=cp4, in1=ohv,
                                    op=ALU.mult)
            nc.vector.tensor_reduce(out=cols[:, 3:4], in_=junk[:, :64],
                                    axis=AX.X, op=ALU.add)

            # ---- intersection grid ----
            for blk, (i0, j0, N, first) in enumerate(sched):
                q12 = psum.tile([128, 512], f32, tag="q12")
                q34 = psum.tile([128, 512], f32, tag="q34")
                nc.tensor.matmul(q12[:, :N], fA12[:, i0:i0 + 128],
                                 fB12[:, j0:j0 + N], start=True, stop=True)
                nc.tensor.matmul(q34[:, :N], fA34[:, i0:i0 + 128],
                                 fB34[:, j0:j0 + N], start=True, stop=True)
                s1 = sig.tile([128, 512], f32, tag="s1")
                s2 = sig.tile([128, 512], f32, tag="s2")
                nc.scalar.activation(out=s1[:, :N], in_=q12[:, :N],
                                     func=ACT.Sigmoid, scale=-0.01)
                nc.scalar.activation(out=s2[:, :N], in_=q34[:, :N],
                                     func=ACT.Sigmoid, scale=-0.01)
                if first:
                    s1m = sig.tile([128, W_DIAG], f32, tag="s1m")
                    nc.vector.tensor_tensor(
                        out=s1m[:, :N], in0=s1[:, :N],
                        in1=tri[:, :N], op=ALU.mult)
                    in0 = s1m
                else:
                    in0 = s1
                prod = sig.tile([128, 512], f32, tag="prod")
                nc.vector.tensor_tensor(out=prod[:, :N], in0=in0[:, :N],
                                        in1=s2[:, :N], op=ALU.mult)
                c = ISECT_COL0 + blk
                nc.vector.tensor_reduce(out=cols[:, c:c + 1],
                                        in_=prod[:, :N],
                                        axis=AX.X, op=ALU.add)

            nc.sync.dma_start(out=d_out[:], in_=cols[:])

    _split_multi_waits(nc)
    return nc


def kernel(point_pred, orient_pred, class_pred, target_seq, padding_mask):
    pp = np.ascontiguousarray(np.asarray(point_pred, dtype=np.float32))
    op = np.ascontiguousarray(np.asarray(orient_pred, dtype=np.float32))
    cp = np.ascontiguousarray(np.asarray(class_pred, dtype=np.float32))
    ts = np.ascontiguousarray(np.asarray(target_seq, dtype=np.float32))
    pm = np.ascontiguousarray(np.asarray(padding_mask)).astype(bool)

    tc_cls, tp, to, valid, nn, per_core, n_segs = _host_prep(pp, op, cp, ts, pm)

    # grid bound: pad to a multiple of 128 so every band is full-height
    L = max(128, -(-max(n_segs) // 128) * 128)
    L = min(L, -(-M // 128) * 128)  # ceil(M/128)*128 = 2048 max

    if L not in _CACHE:
        _CACHE[L] = _build_program(L)
    nc = _CACHE[L]

    # triangular mask for the leading chunk of each band: keep j-i >= 2
    ii = np.arange(128)[:, None]
    jj = np.arange(W_DIAG)[None, :]
    tri = (jj >= ii + 2).astype(np.float32)

    eye = np.eye(NUM_CLASSES, dtype=np.float32)
    in_maps = []
    for b in range(B):
        pc = per_core[b]
        featpk = np.zeros((8, 4 * L), np.float32)
        w = min(M, L)
        featpk[:8, 0 * L:0 * L + w] = pc["fA12"][:, :w]
        featpk[:8, 1 * L:1 * L + w] = pc["fB12"][:, :w]
        featpk[:6, 2 * L:2 * L + w] = pc["fA34"][:, :w]
        featpk[:6, 3 * L:3 * L + w] = pc["fB34"][:, :w]
        if L > M:
            # mask the padded tail via the inv/ones rows of fA12/fB12
            featpk[6, 0 * L + M:1 * L] = BIG   # fA12 row6 = BIG*inv_i
            featpk[7, 0 * L + M:1 * L] = 1.0   # fA12 row7 = ones
            featpk[6, 1 * L + M:2 * L] = 1.0   # fB12 row6 = ones
            featpk[7, 1 * L + M:2 * L] = BIG   # fB12 row7 = BIG*inv_j

        nf = nn[b].astype(np.float32)[:, None]
        vfb = valid[b].astype(np.float32)
        tokpk = np.concatenate([
            (pp[b] * nf).reshape(128, 32),
            (tp[b] * nf).reshape(128, 32),
            op[b].reshape(128, 32),
            (to[b] * nf).reshape(128, 32),
            cp[b].reshape(128, 64),
            (eye[tc_cls[b]] * vfb[:, None]).reshape(128, 64),
            vfb.reshape(128, 16),
        ], axis=1).astype(np.float32)
        in_maps.append({
            "feat": np.ascontiguousarray(featpk),
            "tri": tri,
            "tok": np.ascontiguousarray(tokpk),
        })

    from concourse.bass_utils import run_bass_kernel_spmd
    global LAST_RESULTS
    kw = dict(TRACE_KWARGS) if TRACE_KWARGS else {}
    res = run_bass_kernel_spmd(nc, in_maps, core_ids=list(range(NCORES)), **kw)
    LAST_RESULTS = res
    parts = [r["partials"] for r in res.results]  # each [128, NCOLS] f32

    f32 = np.float32
    pt_raw = f32(0); cos_sum = f32(0); lse_sum = f32(0); sel_sum = f32(0)
    isect_sum = f32(0)
    for b in range(B):
        p = parts[b].astype(np.float32)
        pt_raw += p[:, 0].sum(dtype=np.float32)
        cos_sum += p[:, 1].sum(dtype=np.float32)
        lse_sum += p[:, 2].sum(dtype=np.float32)
        sel_sum += p[:, 3].sum(dtype=np.float32)
        isect_sum += p[:, ISECT_COL0:].sum(dtype=np.float32)

    # wrap-pair exclusion + pair count (host, exact)
    wrap_sum = np.float64(0.0)
    cnt_total = 0
    for b in range(B):
        pc = per_core[b]
        n, n_seg = pc["n"], pc["n_seg"]
        if n < 4:
            continue
        cnt_total += (n_seg - 1) * (n_seg - 2) // 2 - 1
        jw = n_seg - 1
        q12w = np.float32(
            np.dot(pc["fA12"][:, 0].astype(np.float32),
                   pc["fB12"][:, jw].astype(np.float32)))
        q34w = np.float32(
            np.dot(pc["fA34"][:, 0].astype(np.float32),
                   pc["fB34"][:, jw].astype(np.float32)))
        with np.errstate(over="ignore"):
            sw = (1.0 / (1.0 + np.exp(np.float64(0.01) * q12w)) *
                  1.0 / (1.0 + np.exp(np.float64(0.01) * q34w)))
        wrap_sum += sw

    valid_cnt = f32(valid.sum())
    nn_cnt = f32(nn.sum())
    vden = max(valid_cnt, f32(1.0))
    nden = max(nn_cnt, f32(1.0))

    pt_loss = f32(pt_raw * f32(0.25 / (RETINA * RETINA)) / nden)
    orient_loss = f32((nn_cnt - cos_sum) / nden)
    cls_loss = f32((lse_sum - sel_sum) / vden)
    if cnt_total > 0:
        isect_loss = f32((np.float64(isect_sum) - wrap_sum) / cnt_total)
    else:
        isect_loss = f32(0.0)
    total = f32(pt_loss + f32(0.5) * orient_loss + cls_loss
                + f32(0.1) * isect_loss)
    return (total, pt_loss, orient_loss, cls_loss, isect_loss)
